# revision 29
# baseline (speedup 1.0000x reference)
"""Multi-head attention forward on 8 TRN2 NeuronCores.

Problem: x[2,2048,1024] @ {Wq,Wk,Wv}[1024,1024] (+bias) -> 16 heads of 64,
softmax(QK^T/8)V per head, concat -> @Wo[1024,1024] + bo.

Sharding: tensor-parallel over d_hid. Core c owns 2 heads (128 dims):
  - computes Q^T,K^T,V^T slices [128, 2048] per batch from full x^T
  - attention for its (2 batches x 2 heads)
  - partial out = ctx_slice @ Wo[slice_rows] -> [4096, 1024] (f16)
Host sums the 8 partials and adds bo (pure reduction, no collectives).

Key layout/perf decisions:
  - x^T [1024, 4096] uploaded pre-transposed, f16, loaded ONCE for both
    batches into a single [128, 8, BS] SBUF tile (8MB) via 1MB strip
    DMAs on the sync queue; weights ride the gpsimd queue (first-use
    order) so only wq gates the first projection; the scalar queue
    stays free for exp.
  - Everything f16 on the PE (no fp32/fp32r matmuls — they are slower
    and periodically trip the HAM 0.5x utilization throttle).
  - The 1/8 softmax scale is folded into the Exp activation's scale
    operand (not the weights) so Q keeps full f16 range.
  - Scores computed transposed (S^T[k,q]); softmax normalization comes
    from a ones-column augmented V (row 64 of the ctx psum = row sums).
  - Attention runs a flat software pipeline over (qh, kp): the ctx
    accumulation lags scores/exp by one k-pair, and per-qh epilogue
    work (normalization phase 2 + that q-chunk's out-projection tiles)
    is queued as deferred units drained one per kp step — also across
    the batch boundary — so the in-order tensor/DVE queues never park
    and there is no serial tail.
  - Normalization: stage ctx psum to SBUF + f16 sums row (DVE), then a
    deferred 1-partition f16 matmul broadcasts the sums across 64
    partitions into PSUM, reciprocal_approx_fast + multiply on DVE.
  - out partials staged as f16, written with merged 2KB-row DMAs; the
    final drain splits copies across DVE/scalar and DMAs to gpsimd.
  - PSUM banks: scores h0/h1 (2+2) + ctx h0/h1 (1+1) + shared pool for
    proj/transpose/outproj/broadcast (2) = 8.
  - fp8 was evaluated for scores/ctx/V (DoubleRow, 2x PE rate) but the
    attention output is cancellation-dominated, so fp8 quantization
    noise passes through at full relative magnitude (~1-2e-2) — over
    the error budget; sub-128-partition DoubleRow matmuls also run at
    partition-proportional (degraded) rate on HW.
"""

import os
import numpy as np

B, S, D = 2, 2048, 1024
NCORES = 8
HSLICE = D // NCORES          # 128 = 2 heads x 64
KT_PROJ = D // 128            # 8 contraction tiles for projections
NKT = S // 128                # 16 k-tiles per batch for attention
NKP = NKT // 2                # 8 k-tile pairs
QH = 512                      # q chunk (1 PSUM bank)
CH = 512                      # matmul free-dim chunk
BS = B * S

_cache = {}


def _build():
    import concourse.bacc as bacc
    import concourse.tile as tile
    from concourse import mybir

    f32 = mybir.dt.float32
    f16 = mybir.dt.float16
    AF = mybir.ActivationFunctionType

    nc = bacc.Bacc("TRN2", target_bir_lowering=False, debug=False,
                   num_devices=NCORES)

    xt_d = nc.dram_tensor("xt", [D, BS], f16, kind="ExternalInput").ap()
    wq_d = nc.dram_tensor("wq", [128, KT_PROJ * HSLICE], f16, kind="ExternalInput").ap()
    wkv_d = nc.dram_tensor("wkv", [128, 2 * KT_PROJ * HSLICE], f16, kind="ExternalInput").ap()
    b3_d = nc.dram_tensor("b3", [HSLICE, 3], f32, kind="ExternalInput").ap()
    wo_d = nc.dram_tensor("wo", [HSLICE, D], f16, kind="ExternalInput").ap()
    idt_d = nc.dram_tensor("idt", [128, 128], f16, kind="ExternalInput").ap()
    out_d = nc.dram_tensor("out", [BS, D], f16, kind="ExternalOutput").ap()

    with tile.TileContext(nc) as tc:
        with (
            tc.tile_pool(name="wpool", bufs=1) as wpool,
            tc.tile_pool(name="xt", bufs=1) as xtp,
            tc.tile_pool(name="qk", bufs=2) as qkp,
            tc.tile_pool(name="vtmp", bufs=2) as vtp,
            tc.tile_pool(name="vaug", bufs=2) as vap,
            tc.tile_pool(name="et", bufs=2) as etp,
            tc.tile_pool(name="ctx", bufs=2) as ctxp,
            tc.tile_pool(name="norm", bufs=2) as normp,
            tc.tile_pool(name="ost", bufs=3) as ostp,
            tc.tile_pool(name="psS", bufs=1, space="PSUM") as psS,
            tc.tile_pool(name="psC", bufs=1, space="PSUM") as psC,
            tc.tile_pool(name="psP", bufs=2, space="PSUM") as psP,
        ):
            # ---- constants / weights: only wq/b3 gate the start; the
            # rest streams in between x strips on the gpsimd queue ----
            b3_t = wpool.tile([128, 3], f32, tag="b3")
            nc.gpsimd.dma_start(b3_t[:], b3_d[:])
            bq_t, bk_t, bv_t = b3_t[:, 0:1], b3_t[:, 1:2], b3_t[:, 2:3]
            wq_t = wpool.tile([128, KT_PROJ, HSLICE], f16, tag="wq")
            nc.gpsimd.dma_start(wq_t[:].rearrange("p a b -> p (a b)"), wq_d[:])
            wkv_t = wpool.tile([128, 2, KT_PROJ, HSLICE], f16, tag="wkv")
            wk_t, wv_t = wkv_t[:, 0], wkv_t[:, 1]
            wo_t = wpool.tile([128, D], f16, tag="wo")
            idt = wpool.tile([128, 128], f16, tag="idt")

            # ---- load x^T for BOTH batches as ONE tile [128, 8, BS]:
            # one 1MB strip DMA per 512 columns, all on the sync queue
            # (weights ride the gpsimd queue in parallel; the scalar
            # queue stays free for exp) ----
            xt_all = xtp.tile([128, KT_PROJ, BS], f16, tag="xta")
            xts = [xt_all[:, ki, :] for ki in range(KT_PROJ)]
            XC = 512
            xt_dv = xt_d.rearrange("(a p) s -> p a s", p=128)
            for c in range(BS // XC):
                nc.sync.dma_start(
                    xt_all[:, :, c * XC:(c + 1) * XC],
                    xt_dv[:, :, c * XC:(c + 1) * XC])
                if c == 0:
                    nc.gpsimd.dma_start(
                        wkv_t[:].rearrange("p a b c -> p (a b c)"), wkv_d[:])
                elif c == 1:
                    nc.gpsimd.dma_start(idt[:], idt_d[:])
                elif c == 2:
                    nc.gpsimd.dma_start(wo_t[:], wo_d[:])

            pending = []
            last_drain = [False]
            for b in range(B):
                s0 = b * S
                # ---- projections ----
                qt0 = qkp.tile([128, S], f16, tag="qt0")
                qt1 = qkp.tile([128, S], f16, tag="qt1")
                qth = [qt0, qt1]
                nc.vector.memset(qt0[64:128, :], 0.0)
                nc.vector.memset(qt1[0:64, :], 0.0)
                kt = qkp.tile([128, S], f16, tag="kt")
                vt = vtp.tile([128, S], f16, tag="vt")
                for c in range(S // CH):
                    for di, (dst, w_t, b_t) in enumerate(
                            ((None, wq_t, bq_t), (kt, wk_t, bk_t), (vt, wv_t, bv_t))):
                        ps = psP.tile([128, CH], f32, tag="pp")
                        for ki in range(KT_PROJ):
                            nc.tensor.matmul(ps[:], w_t[:, ki, :],
                                             xts[ki][:, s0 + c * CH:s0 + (c + 1) * CH],
                                             start=(ki == 0),
                                             stop=(ki == KT_PROJ - 1))
                        if dst is None:
                            nc.vector.tensor_scalar_add(
                                qt0[0:64, c * CH:(c + 1) * CH],
                                ps[0:64, :], b_t[0:64, :])
                            nc.vector.tensor_scalar_add(
                                qt1[64:128, c * CH:(c + 1) * CH],
                                ps[64:128, :], b_t[64:128, :])
                        else:
                            nc.vector.tensor_scalar_add(
                                dst[:, c * CH:(c + 1) * CH], ps[:], b_t[:, :])

                # ---- V^T -> V_aug pair tiles [128, 2, 130] ----
                vaugs = []
                for kp in range(NKP):
                    va = vap.tile([128, 2, 130], f16, tag=f"va{kp}")
                    for j in range(2):
                        ki = 2 * kp + j
                        pst = psP.tile([128, 128], f16, tag="pp")
                        nc.tensor.transpose(pst[:],
                                            vt[:, ki * 128:(ki + 1) * 128],
                                            idt[:])
                        nc.vector.tensor_copy(va[:, j, 0:64], pst[:, 0:64])
                        nc.vector.tensor_copy(va[:, j, 65:129], pst[:, 64:128])
                    nc.vector.memset(va[:, :, 64:65], 1.0)
                    nc.vector.memset(va[:, :, 129:130], 1.0)
                    vaugs.append(va)

                # ---- attention: flat (qh, kp) software pipeline with
                # deferred norm-phase2 / out-projection work drained one
                # unit per kp step (keeps every queue busy, no tail) ----
                ctxT = ctxp.tile([128, S], f16, tag="ctxT")

                def outproj_st(st, s0=s0, ctxT=ctxT):
                    ot = ostp.tile([128, D], f16, tag="ost")
                    for c2 in range(D // CH):
                        ps = psP.tile([128, CH], f32, tag="pp")
                        nc.tensor.matmul(ps[:],
                                         ctxT[:, st * 128:(st + 1) * 128],
                                         wo_t[:, c2 * CH:(c2 + 1) * CH])
                        if last_drain[0] and c2 == 1:
                            nc.scalar.copy(ot[:, c2 * CH:(c2 + 1) * CH],
                                           ps[:])
                        else:
                            nc.vector.tensor_copy(
                                ot[:, c2 * CH:(c2 + 1) * CH], ps[:])
                    eng = nc.gpsimd if last_drain[0] else nc.sync
                    eng.dma_start(
                        out_d[s0 + st * 128:s0 + (st + 1) * 128, :], ot[:])

                def ctx_step(ctx_ps, vaugs_, kp, ets):
                    for h in range(2):
                        for j in range(2):
                            ki = 2 * kp + j
                            nc.tensor.matmul(
                                ctx_ps[h][:],
                                vaugs_[kp][:, j, h * 65:h * 65 + 65],
                                ets[h][:, j, :],
                                start=(ki == 0), stop=(ki == NKT - 1))

                def norm_phase1(ctx_ps, q0):
                    # stage psum data rows to SBUF f32 and the sums row to a
                    # partition-0 f32 row tile (DVE only; frees ctx banks).
                    stgs, rs = [], []
                    for h in range(2):
                        stg = normp.tile([128, QH], f32, tag=f"stg{h}")
                        nc.vector.tensor_copy(stg[0:64, :], ctx_ps[h][0:64, :])
                        stgs.append(stg)
                    for h in range(2):
                        r = normp.tile([1, QH], f32, tag=f"r{h}")
                        nc.vector.tensor_copy(r[0:1, :], ctx_ps[h][64:65, :])
                        rs.append(r)
                    return stgs, rs

                def norm_phase2a(stgs, rs, q0, norm_phase2b=None):
                    # broadcast the sums rows across 64 partitions on the
                    # (otherwise idle) gpsimd engine; the DVE work that
                    # consumes them is deferred one more step so the gpsimd
                    # latency never blocks the in-order DVE queue.
                    bcss = []
                    for h in range(2):
                        bcs = normp.tile([64, QH], f32, tag=f"bcs{h}")
                        nc.gpsimd.partition_broadcast(bcs[:], rs[h][0:1, :])
                        bcss.append(bcs)
                    pending.append(lambda st=stgs, bb=bcss, qq=q0,
                                   f=norm_phase2b: f(st, bb, qq))

                def norm_phase2b(stgs, bcss, q0, ctxT=ctxT,
                                 outproj_st=outproj_st):
                    # reciprocal + scale ctx rows (DVE), then enqueue this
                    # q-chunk's out-projection tiles as deferred work.
                    for h in range(2):
                        bc = normp.tile([64, QH], f32, tag=f"bc{h}")
                        nc.vector.reciprocal_approx_fast(bc[:], bcss[h][:])
                        nc.vector.tensor_mul(
                            out=ctxT[h * 64:h * 64 + 64, q0:q0 + QH],
                            in0=stgs[h][0:64, :], in1=bc[:])
                    for stq in range(QH // 128):
                        pending.append(
                            lambda s_=q0 // 128 + stq, f=outproj_st: f(s_))

                prev = None        # (ctx_ps, vaugs, kp, ets, q0)
                for qh in range(S // QH):
                    q0 = qh * QH
                    ctx_ps0 = psC.tile([65, QH], f32, tag="ctx0")
                    ctx_ps1 = psC.tile([65, QH], f32, tag="ctx1")
                    ctx_ps = [ctx_ps0, ctx_ps1]
                    for kp in range(NKP):
                        scs, ets = [], []
                        for h in range(2):
                            sc = psS.tile([128, 2 * QH], f32, tag=f"sc{h}")
                            for j in range(2):
                                ki = 2 * kp + j
                                nc.tensor.matmul(
                                    sc[:, j * QH:(j + 1) * QH],
                                    kt[:, ki * 128:(ki + 1) * 128],
                                    qth[h][:, q0:q0 + QH])
                            scs.append(sc)
                        for h in range(2):
                            et = etp.tile([128, 2, QH], f16, tag=f"et{h}")
                            nc.scalar.activation(
                                et[:].rearrange("p a b -> p (a b)"),
                                scs[h][:], AF.Exp, scale=0.125)
                            ets.append(et)
                        if pending:
                            pending.pop(0)()
                        if prev is not None:
                            ctx_step(*prev[:4])
                            if prev[2] == NKP - 1:
                                stgs, rs = norm_phase1(prev[0], prev[4])
                                pending.append(
                                    lambda st=stgs, rr=rs, qq=prev[4],
                                    f=norm_phase2a, g=norm_phase2b:
                                    f(st, rr, qq, g))
                        prev = (ctx_ps, vaugs, kp, ets, q0)
                # flush this batch's last ctx/norm; leftover deferred units
                # drain inside the next batch's attention (or below if last)
                ctx_step(*prev[:4])
                stgs, rs = norm_phase1(prev[0], prev[4])
                pending.append(lambda st=stgs, rr=rs, qq=prev[4],
                               f=norm_phase2a, g=norm_phase2b:
                               f(st, rr, qq, g))
                prev = None
            last_drain[0] = True
            while pending:
                pending.pop(0)()

    nc.compile()
    return nc


def _get_nc():
    if "nc" not in _cache:
        _cache["nc"] = _build()
    return _cache["nc"]


def _tile_w(w):
    # [1024, 128] -> [128, 8, 128] (partition-major tiles)
    return w.reshape(KT_PROJ, 128, HSLICE).transpose(1, 0, 2)


def _in_maps(x, Wq, bq, Wk, bk, Wv, bv, Wo):
    x = np.ascontiguousarray(np.asarray(x, dtype=np.float32))
    xt = np.ascontiguousarray(x.reshape(BS, D).T).astype(np.float16)

    in_maps = []
    for c in range(NCORES):
        sl = slice(c * HSLICE, (c + 1) * HSLICE)
        wq = np.asarray(Wq, np.float32)[:, sl].astype(np.float16)
        wk = np.asarray(Wk, np.float32)[:, sl].astype(np.float16)
        wv = np.asarray(Wv, np.float32)[:, sl].astype(np.float16)
        wkv = np.ascontiguousarray(
            np.stack([_tile_w(wk), _tile_w(wv)], axis=1).reshape(128, -1))
        b3 = np.stack([np.asarray(bq, np.float32)[sl],
                       np.asarray(bk, np.float32)[sl],
                       np.asarray(bv, np.float32)[sl]], axis=1)
        in_maps.append({
            "xt": xt,
            "wq": np.ascontiguousarray(_tile_w(wq).reshape(128, -1)),
            "wkv": wkv,
            "b3": np.ascontiguousarray(b3),
            "wo": np.ascontiguousarray(np.asarray(Wo, np.float32)[sl, :]).astype(np.float16),
            "idt": np.eye(128, dtype=np.float16),
        })
    return in_maps


def kernel(x, Wq, bq, Wk, bk, Wv, bv, Wo, bo):
    from concourse.bass_utils import run_bass_kernel_spmd

    nc = _get_nc()
    in_maps = _in_maps(x, Wq, bq, Wk, bk, Wv, bv, Wo)

    res = run_bass_kernel_spmd(nc, in_maps, core_ids=list(range(NCORES)),
                               trace=bool(int(os.environ.get("KTRACE", "0"))))
    _cache["last_result"] = res
    acc = res.results[0]["out"].astype(np.float32)
    for c in range(1, NCORES):
        acc += res.results[c]["out"].astype(np.float32)
    acc += np.asarray(bo, np.float32)[None, :]
    return acc.reshape(B, S, D)


# revision 31
# speedup vs baseline: 1.0206x; 1.0206x over previous
"""Multi-head attention forward on 8 TRN2 NeuronCores.

Problem: x[2,2048,1024] @ {Wq,Wk,Wv}[1024,1024] (+bias) -> 16 heads of 64,
softmax(QK^T/8)V per head, concat -> @Wo[1024,1024] + bo.

Sharding: tensor-parallel over d_hid. Core c owns 2 heads (128 dims):
  - computes Q^T,K^T,V^T slices [128, 2048] per batch from full x^T
  - attention for its (2 batches x 2 heads)
  - partial out = ctx_slice @ Wo[slice_rows] -> [4096, 1024] (f16)
Host sums the 8 partials and adds bo (pure reduction, no collectives).

Key layout/perf decisions:
  - x^T [1024, 4096] uploaded pre-transposed, f16, loaded ONCE for both
    batches into a single [128, 8, BS] SBUF tile (8MB) via 1MB strip
    DMAs on the sync queue; weights ride the gpsimd queue (first-use
    order) so only wq gates the first projection; the scalar queue
    stays free for exp.
  - Everything f16 on the PE (no fp32/fp32r matmuls — they are slower
    and periodically trip the HAM 0.5x utilization throttle).
  - The 1/8 softmax scale is folded into the Exp activation's scale
    operand (not the weights) so Q keeps full f16 range.
  - Scores computed transposed (S^T[k,q]); softmax normalization comes
    from a ones-column augmented V (row 64 of the ctx psum = row sums).
  - Attention runs a flat software pipeline over (qh, kp): the ctx
    accumulation lags scores/exp by one k-pair, and per-qh epilogue
    work (normalization phase 2 + that q-chunk's out-projection tiles)
    is queued as deferred units drained one per kp step — also across
    the batch boundary — so the in-order tensor/DVE queues never park
    and there is no serial tail.
  - Normalization: stage ctx psum to SBUF + f16 sums row (DVE), then a
    deferred 1-partition f16 matmul broadcasts the sums across 64
    partitions into PSUM, reciprocal_approx_fast + multiply on DVE.
  - out partials staged as f16, written with merged 2KB-row DMAs; the
    final drain splits copies across DVE/scalar and DMAs to gpsimd.
  - PSUM banks: scores h0/h1 (2+2) + ctx h0/h1 (1+1) + shared pool for
    proj/transpose/outproj/broadcast (2) = 8.
  - fp8 was evaluated for scores/ctx/V (DoubleRow, 2x PE rate) but the
    attention output is cancellation-dominated, so fp8 quantization
    noise passes through at full relative magnitude (~1-2e-2) — over
    the error budget; sub-128-partition DoubleRow matmuls also run at
    partition-proportional (degraded) rate on HW.
"""

import os
import numpy as np

B, S, D = 2, 2048, 1024
NCORES = 8
HSLICE = D // NCORES          # 128 = 2 heads x 64
KT_PROJ = D // 128            # 8 contraction tiles for projections
NKT = S // 128                # 16 k-tiles per batch for attention
NKP = NKT // 2                # 8 k-tile pairs
QH = 512                      # q chunk (1 PSUM bank)
CH = 512                      # matmul free-dim chunk
BS = B * S

_cache = {}


def _build():
    import concourse.bacc as bacc
    import concourse.tile as tile
    from concourse import mybir

    f32 = mybir.dt.float32
    f16 = mybir.dt.float16
    AF = mybir.ActivationFunctionType

    nc = bacc.Bacc("TRN2", target_bir_lowering=False, debug=False,
                   num_devices=NCORES)

    xt_d = nc.dram_tensor("xt", [D, BS], f16, kind="ExternalInput").ap()
    wq_d = nc.dram_tensor("wq", [128, KT_PROJ * HSLICE], f16, kind="ExternalInput").ap()
    wkv_d = nc.dram_tensor("wkv", [128, 2 * KT_PROJ * HSLICE], f16, kind="ExternalInput").ap()
    b3_d = nc.dram_tensor("b3", [HSLICE, 3], f32, kind="ExternalInput").ap()
    wo_d = nc.dram_tensor("wo", [HSLICE, D], f16, kind="ExternalInput").ap()
    idt_d = nc.dram_tensor("idt", [128, 128], f16, kind="ExternalInput").ap()
    out_d = nc.dram_tensor("out", [BS, D], f16, kind="ExternalOutput").ap()

    with tile.TileContext(nc) as tc:
        with (
            tc.tile_pool(name="wpool", bufs=1) as wpool,
            tc.tile_pool(name="xt", bufs=1) as xtp,
            tc.tile_pool(name="qk", bufs=2) as qkp,
            tc.tile_pool(name="vtmp", bufs=2) as vtp,
            tc.tile_pool(name="vaug", bufs=2) as vap,
            tc.tile_pool(name="et", bufs=2) as etp,
            tc.tile_pool(name="ctx", bufs=2) as ctxp,
            tc.tile_pool(name="norm", bufs=2) as normp,
            tc.tile_pool(name="ost", bufs=3) as ostp,
            tc.tile_pool(name="psS", bufs=1, space="PSUM") as psS,
            tc.tile_pool(name="psC", bufs=1, space="PSUM") as psC,
            tc.tile_pool(name="psP", bufs=2, space="PSUM") as psP,
        ):
            # ---- constants / weights: only wq/b3 gate the start; the
            # rest streams in between x strips on the gpsimd queue ----
            b3_t = wpool.tile([128, 3], f32, tag="b3")
            nc.gpsimd.dma_start(b3_t[:], b3_d[:])
            bq_t, bk_t, bv_t = b3_t[:, 0:1], b3_t[:, 1:2], b3_t[:, 2:3]
            wq_t = wpool.tile([128, KT_PROJ, HSLICE], f16, tag="wq")
            nc.gpsimd.dma_start(wq_t[:].rearrange("p a b -> p (a b)"), wq_d[:])
            wkv_t = wpool.tile([128, 2, KT_PROJ, HSLICE], f16, tag="wkv")
            wk_t, wv_t = wkv_t[:, 0], wkv_t[:, 1]
            wo_t = wpool.tile([128, D], f16, tag="wo")
            idt = wpool.tile([128, 128], f16, tag="idt")
            ones_t = wpool.tile([128, 64], f16, tag="ones")
            nc.vector.memset(ones_t[:], 1.0)

            # ---- load x^T for BOTH batches as ONE tile [128, 8, BS]:
            # one 1MB strip DMA per 512 columns, all on the sync queue
            # (weights ride the gpsimd queue in parallel; the scalar
            # queue stays free for exp) ----
            xt_all = xtp.tile([128, KT_PROJ, BS], f16, tag="xta")
            xts = [xt_all[:, ki, :] for ki in range(KT_PROJ)]
            XC = 512
            xt_dv = xt_d.rearrange("(a p) s -> p a s", p=128)
            for c in range(BS // XC):
                nc.sync.dma_start(
                    xt_all[:, :, c * XC:(c + 1) * XC],
                    xt_dv[:, :, c * XC:(c + 1) * XC])
                if c == 0:
                    nc.gpsimd.dma_start(
                        wkv_t[:].rearrange("p a b c -> p (a b c)"), wkv_d[:])
                elif c == 1:
                    nc.gpsimd.dma_start(idt[:], idt_d[:])
                elif c == 2:
                    nc.gpsimd.dma_start(wo_t[:], wo_d[:])

            pending = []
            last_drain = [False]
            for b in range(B):
                s0 = b * S
                # ---- projections ----
                qt0 = qkp.tile([128, S], f16, tag="qt0")
                qt1 = qkp.tile([128, S], f16, tag="qt1")
                qth = [qt0, qt1]
                nc.vector.memset(qt0[64:128, :], 0.0)
                nc.vector.memset(qt1[0:64, :], 0.0)
                kt = qkp.tile([128, S], f16, tag="kt")
                vt = vtp.tile([128, S], f16, tag="vt")
                for c in range(S // CH):
                    for di, (dst, w_t, b_t) in enumerate(
                            ((None, wq_t, bq_t), (kt, wk_t, bk_t), (vt, wv_t, bv_t))):
                        ps = psP.tile([128, CH], f32, tag="pp")
                        for ki in range(KT_PROJ):
                            nc.tensor.matmul(ps[:], w_t[:, ki, :],
                                             xts[ki][:, s0 + c * CH:s0 + (c + 1) * CH],
                                             start=(ki == 0),
                                             stop=(ki == KT_PROJ - 1))
                        if dst is None:
                            nc.vector.tensor_scalar_add(
                                qt0[0:64, c * CH:(c + 1) * CH],
                                ps[0:64, :], b_t[0:64, :])
                            nc.vector.tensor_scalar_add(
                                qt1[64:128, c * CH:(c + 1) * CH],
                                ps[64:128, :], b_t[64:128, :])
                        else:
                            nc.vector.tensor_scalar_add(
                                dst[:, c * CH:(c + 1) * CH], ps[:], b_t[:, :])

                # ---- V^T -> V_aug pair tiles [128, 2, 130] ----
                vaugs = []
                for kp in range(NKP):
                    va = vap.tile([128, 2, 130], f16, tag=f"va{kp}")
                    for j in range(2):
                        ki = 2 * kp + j
                        pst = psP.tile([128, 128], f16, tag="pp")
                        nc.tensor.transpose(pst[:],
                                            vt[:, ki * 128:(ki + 1) * 128],
                                            idt[:])
                        nc.vector.tensor_copy(va[:, j, 0:64], pst[:, 0:64])
                        nc.vector.tensor_copy(va[:, j, 65:129], pst[:, 64:128])
                    nc.vector.memset(va[:, :, 64:65], 1.0)
                    nc.vector.memset(va[:, :, 129:130], 1.0)
                    vaugs.append(va)

                # ---- attention: flat (qh, kp) software pipeline with
                # deferred norm-phase2 / out-projection work drained one
                # unit per kp step (keeps every queue busy, no tail) ----
                ctxT = ctxp.tile([128, S], f16, tag="ctxT")

                def outproj_st(st, s0=s0, ctxT=ctxT):
                    ot = ostp.tile([128, D], f16, tag="ost")
                    for c2 in range(D // CH):
                        ps = psP.tile([128, CH], f32, tag="pp")
                        nc.tensor.matmul(ps[:],
                                         ctxT[:, st * 128:(st + 1) * 128],
                                         wo_t[:, c2 * CH:(c2 + 1) * CH])
                        if last_drain[0] and c2 == 1:
                            nc.scalar.copy(ot[:, c2 * CH:(c2 + 1) * CH],
                                           ps[:])
                        else:
                            nc.vector.tensor_copy(
                                ot[:, c2 * CH:(c2 + 1) * CH], ps[:])
                    eng = nc.gpsimd if last_drain[0] else nc.sync
                    eng.dma_start(
                        out_d[s0 + st * 128:s0 + (st + 1) * 128, :], ot[:])

                def ctx_step(ctx_ps, vaugs_, kp, ets):
                    for h in range(2):
                        for j in range(2):
                            ki = 2 * kp + j
                            nc.tensor.matmul(
                                ctx_ps[h][:],
                                vaugs_[kp][:, j, h * 65:h * 65 + 65],
                                ets[h][:, j, :],
                                start=(ki == 0), stop=(ki == NKT - 1))

                def norm_phase1(ctx_ps, q0):
                    # stage psum data rows to SBUF f32 and the sums row to a
                    # partition-0 f32 row tile (DVE only; frees ctx banks).
                    stgs, rs = [], []
                    for h in range(2):
                        stg = normp.tile([128, QH], f32, tag=f"stg{h}")
                        nc.vector.tensor_copy(stg[0:64, :], ctx_ps[h][0:64, :])
                        stgs.append(stg)
                    for h in range(2):
                        r = normp.tile([1, QH], f16, tag=f"r{h}")
                        nc.vector.tensor_copy(r[0:1, :], ctx_ps[h][64:65, :])
                        rs.append(r)
                    return stgs, rs

                def norm_phase2(stgs, rs, q0, ctxT=ctxT,
                                outproj_st=outproj_st):
                    # broadcast the sums row across 64 partitions via a
                    # 1-partition f16 matmul, reciprocal on the psum result,
                    # then scale ctx rows (DVE). Enqueues this q-chunk's
                    # out-projection tiles as deferred work.
                    for h in range(2):
                        bcps = psP.tile([64, QH], f32, tag="pp")
                        nc.tensor.matmul(bcps[:], ones_t[0:1, :],
                                         rs[h][0:1, :])
                        bc = normp.tile([64, QH], f32, tag=f"bc{h}")
                        nc.vector.reciprocal_approx_fast(bc[:], bcps[:])
                        nc.vector.tensor_mul(
                            out=ctxT[h * 64:h * 64 + 64, q0:q0 + QH],
                            in0=stgs[h][0:64, :], in1=bc[:])
                    for stq in range(QH // 128):
                        pending.append(
                            lambda s_=q0 // 128 + stq, f=outproj_st: f(s_))

                prev = None        # (ctx_ps, vaugs, kp, ets, q0)
                for qh in range(S // QH):
                    q0 = qh * QH
                    ctx_ps0 = psC.tile([65, QH], f32, tag="ctx0")
                    ctx_ps1 = psC.tile([65, QH], f32, tag="ctx1")
                    ctx_ps = [ctx_ps0, ctx_ps1]
                    for kp in range(NKP):
                        scs, ets = [], []
                        for h in range(2):
                            sc = psS.tile([128, 2 * QH], f32, tag=f"sc{h}")
                            for j in range(2):
                                ki = 2 * kp + j
                                nc.tensor.matmul(
                                    sc[:, j * QH:(j + 1) * QH],
                                    kt[:, ki * 128:(ki + 1) * 128],
                                    qth[h][:, q0:q0 + QH])
                            scs.append(sc)
                        for h in range(2):
                            et = etp.tile([128, 2, QH], f16, tag=f"et{h}")
                            nc.scalar.activation(
                                et[:].rearrange("p a b -> p (a b)"),
                                scs[h][:], AF.Exp, scale=0.125)
                            ets.append(et)
                        if pending:
                            pending.pop(0)()
                        if prev is not None:
                            ctx_step(*prev[:4])
                            if prev[2] == NKP - 1:
                                stgs, rs = norm_phase1(prev[0], prev[4])
                                pending.append(
                                    lambda st=stgs, rr=rs, qq=prev[4],
                                    f=norm_phase2: f(st, rr, qq))
                        prev = (ctx_ps, vaugs, kp, ets, q0)
                # flush this batch's last ctx/norm; leftover deferred units
                # drain inside the next batch's attention (or below if last)
                ctx_step(*prev[:4])
                stgs, rs = norm_phase1(prev[0], prev[4])
                pending.append(lambda st=stgs, rr=rs, qq=prev[4],
                               f=norm_phase2: f(st, rr, qq))
                prev = None
            last_drain[0] = True
            while pending:
                pending.pop(0)()

    nc.compile()
    return nc


def _get_nc():
    if "nc" not in _cache:
        _cache["nc"] = _build()
    return _cache["nc"]


def _tile_w(w):
    # [1024, 128] -> [128, 8, 128] (partition-major tiles)
    return w.reshape(KT_PROJ, 128, HSLICE).transpose(1, 0, 2)


def _in_maps(x, Wq, bq, Wk, bk, Wv, bv, Wo):
    x = np.ascontiguousarray(np.asarray(x, dtype=np.float32))
    xt = np.ascontiguousarray(x.reshape(BS, D).T).astype(np.float16)

    in_maps = []
    for c in range(NCORES):
        sl = slice(c * HSLICE, (c + 1) * HSLICE)
        wq = np.asarray(Wq, np.float32)[:, sl].astype(np.float16)
        wk = np.asarray(Wk, np.float32)[:, sl].astype(np.float16)
        wv = np.asarray(Wv, np.float32)[:, sl].astype(np.float16)
        wkv = np.ascontiguousarray(
            np.stack([_tile_w(wk), _tile_w(wv)], axis=1).reshape(128, -1))
        b3 = np.stack([np.asarray(bq, np.float32)[sl],
                       np.asarray(bk, np.float32)[sl],
                       np.asarray(bv, np.float32)[sl]], axis=1)
        in_maps.append({
            "xt": xt,
            "wq": np.ascontiguousarray(_tile_w(wq).reshape(128, -1)),
            "wkv": wkv,
            "b3": np.ascontiguousarray(b3),
            "wo": np.ascontiguousarray(np.asarray(Wo, np.float32)[sl, :]).astype(np.float16),
            "idt": np.eye(128, dtype=np.float16),
        })
    return in_maps


def kernel(x, Wq, bq, Wk, bk, Wv, bv, Wo, bo):
    from concourse.bass_utils import run_bass_kernel_spmd

    nc = _get_nc()
    in_maps = _in_maps(x, Wq, bq, Wk, bk, Wv, bv, Wo)

    res = run_bass_kernel_spmd(nc, in_maps, core_ids=list(range(NCORES)),
                               trace=bool(int(os.environ.get("KTRACE", "0"))))
    _cache["last_result"] = res
    acc = res.results[0]["out"].astype(np.float32)
    for c in range(1, NCORES):
        acc += res.results[c]["out"].astype(np.float32)
    acc += np.asarray(bo, np.float32)[None, :]
    return acc.reshape(B, S, D)
